# revision 48
# baseline (speedup 1.0000x reference)
"""ARCAttention (MLA + pattern-attention + gate) distributed Bass kernel for 8 TRN2 NeuronCores.

Sharding: data-parallel over batch (B=2) x tensor-parallel over heads (4 head-groups).
Core c handles batch (c // 4), heads [4*(c%4) .. 4*(c%4)+4) of both the MLA path and the
pattern path. The low-rank a-projections (q_a, kv_a lora) and the gate are replicated
within a batch group. Each core emits a partial (already gate-weighted) output
[S, HID]; the host sums the 4 partials per batch. No device collectives.

Scheduling notes (v2):
- startup: x / first q_a weight chunk split into per-4k tiles; the first six q_a
  chains advance k-group-wise in a dedicated 6-bank PSUM pool so PE starts ~3us in.
- rmsnorm: Rsqrt activation directly (no DVE reciprocal); the per-token scale
  commutes through the b-projections, so it is applied post-hoc to qnope/qpe and
  in-place on kvnT under cover of the q_b chains. Nothing serializes on it.
- gate + pattern q/k/v projections are emitted as PE filler work inside the MLA
  attention loop (between scores and ctx matmuls), keeping PE busy while
  scalar/vector/gpsimd chew softmax.
- main-path o-proj runs as filler inside pattern attention with the g0 gate scale
  folded into the psum->sbuf copy; stage-5 combine is one scalar_tensor_tensor per
  chunk and tb 0-3 are emitted inside the pattern loop.
"""

import numpy as np
import ml_dtypes

# ---- model config (hardcoded from the problem spec) ----
B, S, HID = 2, 1024, 2048
H = 16
D_NOPE, D_ROPE, D_V = 128, 64, 128
D_Q = D_NOPE + D_ROPE            # 192
QR, KVR = 1536, 512
PH, PD = 16, 128
THETA, EPS = 10000.0, 1e-6
NCORES = 8
HPC = 4                          # heads per core
TB = S // 128                    # 8 token blocks
KT_HID = HID // 128              # 16
KT_QR = QR // 128                # 12
KT_KVR = KVR // 128              # 4

BF16 = ml_dtypes.bfloat16

# knobs for test harness
TRACE = False
RUN_KWARGS = {}
LAST_RESULT = None

_graph_cache = {}


def _build_graph():
    from contextlib import ExitStack
    import concourse.bass as bass
    import concourse.mybir as mybir
    import concourse.tile as tile
    from concourse import bacc, bass_isa

    BF = mybir.dt.bfloat16
    F32 = mybir.dt.float32
    Exp = mybir.ActivationFunctionType.Exp
    Square = mybir.ActivationFunctionType.Square
    Sqrt = mybir.ActivationFunctionType.Sqrt
    Copy = mybir.ActivationFunctionType.Copy
    MULT = mybir.AluOpType.mult
    ADD = mybir.AluOpType.add
    X = mybir.AxisListType.X
    ts = bass.ts

    nc = bacc.Bacc("TRN2", target_bir_lowering=False, debug=False,
                   num_devices=NCORES)

    def din(name, shape, dt=BF):
        return nc.declare_dram_parameter(name, list(shape), dt, isOutput=False)

    xT_d = din("xT", [HID, S])
    qa_d = din("qa_wT", [HID, QR])
    qbn_d = din("qbn_wT", [QR, HPC * D_NOPE])
    qbp_d = din("qbp_wT", [QR, HPC * D_ROPE])
    kvl_d = din("kvl_wT", [HID, KVR])
    kvp_d = din("kvp_wT", [HID, HPC * D_ROPE])
    kbn_d = din("kbn_wT", [KVR, HPC * D_NOPE])
    kbv_d = din("kbv_wT", [KVR, HPC * D_V])
    ow_d = din("o_wT", [HPC * D_V, HID])
    spq_d = din("spq_wT", [HID, HPC * PD])
    spk_d = din("spk_wT", [HID, HPC * PD])
    spv_d = din("spv_wT", [HID, HPC * PD])
    spo_d = din("spo_wT", [HPC * PD, HID])
    gw_d = din("gate_wT", [HID, 2])
    gb_d = din("gate_bias", [2, 1], F32)
    id2_d = din("ident2", [2, 2])
    cos_d = din("cos2T", [128, S])
    sin_d = din("sin2T", [128, S])
    out_d = nc.declare_dram_parameter("out", [S, HID], BF, isOutput=True)

    def r3(dram, kt):
        # [kt*128, N] dram tensor viewed as [128, kt, N] for SBUF tiling
        return dram.ap().rearrange("(k p) n -> p k n", p=128, k=kt)

    es = ExitStack()
    with tile.TileContext(nc) as tc, es:
        constp = es.enter_context(tc.tile_pool(name="const", bufs=1))
        wring = es.enter_context(tc.tile_pool(name="wring", bufs=2))
        ainp = es.enter_context(tc.tile_pool(name="ain_pat", bufs=1))
        ctxp = es.enter_context(tc.tile_pool(name="ctxp", bufs=1))
        # xp is opened after the es-scoped pools and closed right after stage
        # 4a (pools release LIFO), freeing x's 32KB/partition for stage 4b
        es_xp = ExitStack()
        xp = es_xp.enter_context(tc.tile_pool(name="xp", bufs=1))

        eps_t = constp.tile([128, 1], F32, tag="eps")
        nc.vector.memset(eps_t[:], EPS)
        ones_col = constp.tile([128, 1], BF, tag="ones_col")
        nc.vector.memset(ones_col[:], 1.0)
        ones_row = constp.tile([1, 128], BF, tag="ones_row")
        nc.vector.memset(ones_row[:], 1.0)
        ident2 = constp.tile([2, 2], BF, tag="ident2")
        cosT = constp.tile([128, S], BF, tag="cos")
        sinT = constp.tile([128, S], BF, tag="sin")
        gbias = constp.tile([2, 1], F32, tag="gb")
        gwt = constp.tile([128, KT_HID, 2], BF, tag="gw")

        # x split into 4 k-group tiles for fine-grained startup deps
        xTs = [xp.tile([128, 4, S], BF, tag=f"xT{g}", name=f"xT{g}")
               for g in range(4)]

        def xk(k):
            return xTs[k // 4][:, k % 4, :]

        # pattern outputs (written as fillers during MLA attention)
        pqT = ainp.tile([128, HPC, S], BF, tag="pqT")
        pkT = ainp.tile([128, HPC, S], BF, tag="pkT")
        pv_s = ainp.tile([128, TB, HPC * PD], BF, tag="pv")
        # per-token gates broadcast across partitions; folded into the
        # attention ctx normalization (ctxT_m pre-scaled by g0, ctxT_p by g1)
        g0bc = ainp.tile([128, S], BF, tag="g0bc")
        g1bc = ainp.tile([128, S], BF, tag="g1bc")

        ctxT_m = ctxp.tile([128, HPC, S], BF, tag="ctxm")

        # ---- weight ring: rotating 16KB/partition chunks ----
        def ring_chunk(name):
            return wring.tile([128, 8192], BF, tag="w", name=name)

        def kview(ap, k, n):
            return ap.rearrange("p (k n) -> p k n", k=k, n=n)

        def dma4(view, src_r3):
            for kq in range(0, view.shape[1], 4):
                hi = min(kq + 4, view.shape[1])
                nc.sync.dma_start(out=view[:, kq:hi, :], in_=src_r3[:, kq:hi, :])

        # MLA attention inputs (freed after MLA attention)
        es_ain = ExitStack()
        ain = es_ain.enter_context(tc.tile_pool(name="ain_mla", bufs=1))
        qnopeT = ain.tile([128, HPC, S], BF, tag="qnopeT")
        qpeT = ain.tile([128, 2, S], BF, tag="qpeT")
        knopeT = ain.tile([128, HPC, S], BF, tag="knopeT")
        kpeT = ain.tile([128, 2, S], BF, tag="kpeT")
        v_s = ain.tile([128, TB, HPC * D_V], BF, tag="v")   # token-major

        # stage-1/2 scratch (scoped; freed before attention)
        es12 = ExitStack()
        q2 = es12.enter_context(tc.tile_pool(name="q2", bufs=1))
        wk1 = es12.enter_context(tc.tile_pool(name="wk1", bufs=3))
        wrope = es12.enter_context(tc.tile_pool(name="wrope", bufs=1))
        qmidT = q2.tile([128, KT_QR, S], BF, tag="qmidT")
        kvnT = q2.tile([128, KT_KVR, S], BF, tag="kvnT")
        # shared between the q and k rms phases: emit_bcs(q) fully drains
        # these before the kvl stashes re-fill them
        sqacc = [q2.tile([128, 512], F32, tag=f"sqa{i}", name=f"sqa{i}")
                 for i in range(2)]
        # first q_a weight chunk, split per k-group (freed right after boot)
        es_qa0 = ExitStack()
        qa0p = es_qa0.enter_context(tc.tile_pool(name="qa0p", bufs=1))
        wq0s = [qa0p.tile([128, 4, 512], BF, tag=f"qa0_{g}", name=f"qa0_{g}")
                for g in range(4)]
        bcs_q = [q2.tile([128, 512], BF, tag=f"bcsq{i}", name=f"bcsq{i}")
                 for i in range(2)]
        bcs_k = [q2.tile([128, 512], BF, tag=f"bcsk{i}", name=f"bcsk{i}")
                 for i in range(2)]

        # startup DMAs: x spread across four engine queues (per-k slices for
        # the first k-group) so the HBM bandwidth of several DMA queues feeds
        # the boot chains; wq0 chunks go first on the sync queue.
        nc.sync.dma_start(out=wq0s[0][:], in_=r3(qa_d, KT_HID)[:, 0:4, ts(0, 512)])
        for j in range(2):
            nc.scalar.dma_start(out=xTs[0][:, j, :], in_=r3(xT_d, KT_HID)[:, j, :])
        for j in range(2, 4):
            nc.sync.dma_start(out=xTs[0][:, j, :], in_=r3(xT_d, KT_HID)[:, j, :])
        nc.gpsimd.dma_start(out=xTs[1][:], in_=r3(xT_d, KT_HID)[:, 4:8, :])
        nc.scalar.dma_start(out=xTs[2][:], in_=r3(xT_d, KT_HID)[:, 8:12, :])
        nc.gpsimd.dma_start(out=xTs[3][:], in_=r3(xT_d, KT_HID)[:, 12:16, :])
        for g in range(1, 4):
            nc.sync.dma_start(out=wq0s[g][:],
                              in_=r3(qa_d, KT_HID)[:, 4 * g:4 * g + 4, ts(0, 512)])
        wt_qa = [None]
        for ck in (1, 2):
            chq = ring_chunk(f"qa{ck}")
            v = kview(chq, KT_HID, 512)
            dma4(v, r3(qa_d, KT_HID)[:, :, ts(ck, 512)].rearrange("p k n -> p k n"))
            wt_qa.append(v)
        ch_kvl = ring_chunk("kvl")
        wt_kl = kview(ch_kvl, KT_HID, 512)
        dma4(wt_kl, r3(kvl_d, KT_HID))
        ch_kvp = ring_chunk("kvp")
        wt_kp = kview(ch_kvp[:, 0:4096], KT_HID, HPC * D_ROPE)
        nc.sync.dma_start(out=wt_kp[:], in_=r3(kvp_d, KT_HID))
        nc.sync.dma_start(out=gwt[:], in_=r3(gw_d, KT_HID))
        nc.sync.dma_start(out=gbias[:], in_=gb_d.ap())
        nc.sync.dma_start(out=ident2[:], in_=id2_d.ap())
        nc.sync.dma_start(out=cosT[:], in_=cos_d.ap())
        nc.sync.dma_start(out=sinT[:], in_=sin_d.ap())
        ch_w2a = ring_chunk("w2a")
        wqbn = kview(ch_w2a[:, 0:6144], KT_QR, HPC * D_NOPE)
        nc.sync.dma_start(out=wqbn[:], in_=r3(qbn_d, KT_QR))
        wkbn = kview(ch_w2a[:, 6144:8192], KT_KVR, HPC * D_NOPE)
        nc.sync.dma_start(out=wkbn[:], in_=r3(kbn_d, KT_KVR))
        ch_w2b = ring_chunk("w2b")
        wqbp = kview(ch_w2b[:, 0:3072], KT_QR, HPC * D_ROPE)
        nc.sync.dma_start(out=wqbp[:], in_=r3(qbp_d, KT_QR))
        wkbv = kview(ch_w2b[:, 3072:5120], KT_KVR, HPC * D_V)
        nc.sync.dma_start(out=wkbv[:], in_=r3(kbv_d, KT_KVR))
        ch_spq = ring_chunk("spq")
        wspq = kview(ch_spq, KT_HID, HPC * PD)
        dma4(wspq, r3(spq_d, KT_HID))
        ch_spk = ring_chunk("spk")
        wspk = kview(ch_spk, KT_HID, HPC * PD)
        dma4(wspk, r3(spk_d, KT_HID))
        ch_spv = ring_chunk("spv")
        wspv = kview(ch_spv, KT_HID, HPC * PD)
        dma4(wspv, r3(spv_d, KT_HID))
        ch_wo = ring_chunk("wo")
        wo = kview(ch_wo, KT_KVR, HID)
        nc.sync.dma_start(out=wo[:], in_=r3(ow_d, KT_KVR))
        ch_wspo = ring_chunk("wspo")
        wspo = kview(ch_wspo, KT_KVR, HID)
        nc.sync.dma_start(out=wspo[:], in_=r3(spo_d, KT_KVR))

        # sum-of-squares: square on scalar (psum read), accumulate on the
        # otherwise-idle gpsimd engine into per-nck f32 tiles. No PE
        # ones-matmuls, no psum bank for ssq.
        sq_seen = set()

        def stash_sq(ps, nck, phase):
            sq = wk1.tile([128, 512], BF, tag="sq", bufs=5)
            nc.scalar.activation(sq[:], ps[:], Square)
            skey = (phase, nck)
            if skey not in sq_seen:
                sq_seen.add(skey)
                nc.vector.tensor_copy(sqacc[nck][:], sq[:])
            else:
                nc.vector.tensor_add(sqacc[nck][:], sqacc[nck][:], sq[:])

        # rms scale factors: rsqrt(ssq/n + eps) as [128,512] bf16. The
        # partition reduction runs on gpsimd (broadcast output for free),
        # sqrt on scalar, and an approx-fast reciprocal on DVE.
        def emit_bcs(n, dst2):
            for nck in range(2):
                arsb = wrope.tile([128, 512], F32, tag="rot", bufs=1)
                nc.gpsimd.partition_all_reduce(arsb[:], sqacc[nck][:], 128,
                                               bass_isa.ReduceOp.add)
                nc.scalar.activation(sqacc[nck][:], arsb[:], Sqrt,
                                     bias=eps_t[:, 0:1], scale=1.0 / n)
                nc.vector.reciprocal_approx_fast(out=arsb[:], in_=sqacc[nck][:])
                nc.vector.tensor_copy(dst2[nck][:], arsb[:])

        # ================= stage 1 =================
        # boot: first 6 q_a chains (m0-2 x nck0-1) advance k-group-wise so PE
        # starts as soon as the first x / weight k-groups land.
        with tc.tile_pool(name="boot", bufs=6, space="PSUM") as bootp:
            chains6 = [(m, nck) for m in range(3) for nck in range(2)]
            bps = [bootp.tile([128, 512], F32, tag="bp", name=f"bp{i}")
                   for i in range(6)]
            for g in range(4):
                for j in range(4):
                    k = 4 * g + j
                    for ci, (m, nck) in enumerate(chains6):
                        nc.tensor.matmul(bps[ci][:],
                                         lhsT=wq0s[g][:, j, ts(m, 128)],
                                         rhs=xTs[g][:, j, ts(nck, 512)],
                                         start=(k == 0), stop=(k == KT_HID - 1))
            for ci, (m, nck) in enumerate(chains6):
                nc.any.tensor_copy(qmidT[:, m, ts(nck, 512)], bps[ci][:])
                stash_sq(bps[ci], nck, 'q')
            # m3 chains reuse boot slots
            for nck in range(2):
                bp = bootp.tile([128, 512], F32, tag="bp", name=f"bp6_{nck}")
                for g in range(4):
                    for j in range(4):
                        k = 4 * g + j
                        nc.tensor.matmul(bp[:], lhsT=wq0s[g][:, j, ts(3, 128)],
                                         rhs=xTs[g][:, j, ts(nck, 512)],
                                         start=(k == 0), stop=(k == KT_HID - 1))
                nc.any.tensor_copy(qmidT[:, 3, ts(nck, 512)], bp[:])
                stash_sq(bp, nck, 'q')
        es_qa0.close()

        es_pp = ExitStack()
        pp = es_pp.enter_context(tc.tile_pool(name="pp", bufs=3, space="PSUM"))
        pt = es_pp.enter_context(tc.tile_pool(name="pt", bufs=3, space="PSUM"))

        # q_a chunks 1,2
        for ck in (1, 2):
            wt = wt_qa[ck]
            for mm4 in range(4):
                m = ck * 4 + mm4
                for nck in range(2):
                    ps = pp.tile([128, 512], F32, tag="pp")
                    for k in range(KT_HID):
                        nc.tensor.matmul(ps[:], lhsT=wt[:, k, ts(mm4, 128)],
                                         rhs=xk(k)[:, ts(nck, 512)],
                                         start=(k == 0), stop=(k == KT_HID - 1))
                    nc.any.tensor_copy(qmidT[:, m, ts(nck, 512)], ps[:])
                    stash_sq(ps, nck, 'q')

        emit_bcs(QR, bcs_q)

        # kv_a lora part: feature-major [KVR, S]
        for m in range(KT_KVR):
            for nck in range(2):
                ps = pp.tile([128, 512], F32, tag="pp")
                for k in range(KT_HID):
                    nc.tensor.matmul(ps[:], lhsT=wt_kl[:, k, ts(m, 128)],
                                     rhs=xk(k)[:, ts(nck, 512)],
                                     start=(k == 0), stop=(k == KT_HID - 1))
                nc.any.tensor_copy(kvnT[:, m, ts(nck, 512)], ps[:])
                stash_sq(ps, nck, 'k')

        
        def rope_from_psum(ps, dst, nck, work):
            """Apply rope to a [128, 512] psum chunk holding 2 stacked
            64-dim pe heads; write bf16 to dst ([128,512] slice)."""
            rot = work.tile([128, 512], F32, tag="rot")
            nc.vector.tensor_scalar_mul(rot[0:32, :], ps[32:64, :], -1.0)
            nc.vector.tensor_copy(rot[32:64, :], ps[0:32, :])
            nc.vector.tensor_scalar_mul(rot[64:96, :], ps[96:128, :], -1.0)
            nc.vector.tensor_copy(rot[96:128, :], ps[64:96, :])
            nc.vector.tensor_mul(dst, ps[:], cosT[:, ts(nck, 512)])
            nc.vector.tensor_mul(rot[:], rot[:], sinT[:, ts(nck, 512)])
            nc.vector.tensor_add(dst, dst, rot[:])

        # kv_a pe part (2 m-tiles of 2 stacked heads) + rope
        for m in range(2):
            for nck in range(2):
                ps = pt.tile([128, 512], F32, tag="pt")
                for k in range(KT_HID):
                    nc.tensor.matmul(ps[:], lhsT=wt_kp[:, k, ts(m, 128)],
                                     rhs=xk(k)[:, ts(nck, 512)],
                                     start=(k == 0), stop=(k == KT_HID - 1))
                rope_from_psum(ps, kpeT[:, m, ts(nck, 512)], nck, wrope)

        emit_bcs(KVR, bcs_k)

        # ---------- Stage 2: b-projections ----------
        # q_b on RAW qmid; the rms scale is applied post-hoc to qnope/qpe
        # (it commutes through the contraction and rope).
        for h in range(HPC):
            for nck in range(2):
                ps = pt.tile([128, 512], F32, tag="pt")
                for k in range(KT_QR):
                    nc.tensor.matmul(ps[:], lhsT=wqbn[:, k, ts(h, 128)],
                                     rhs=qmidT[:, k, ts(nck, 512)],
                                     start=(k == 0), stop=(k == KT_QR - 1))
                nc.any.tensor_copy(qnopeT[:, h, ts(nck, 512)], ps[:])
                nc.vector.tensor_mul(qnopeT[:, h, ts(nck, 512)],
                                     qnopeT[:, h, ts(nck, 512)], bcs_q[nck][:])
        for m in range(2):
            for nck in range(2):
                ps = pt.tile([128, 512], F32, tag="pt")
                for k in range(KT_QR):
                    nc.tensor.matmul(ps[:], lhsT=wqbp[:, k, ts(m, 128)],
                                     rhs=qmidT[:, k, ts(nck, 512)],
                                     start=(k == 0), stop=(k == KT_QR - 1))
                rope_from_psum(ps, qpeT[:, m, ts(nck, 512)], nck, wrope)
                nc.vector.tensor_mul(qpeT[:, m, ts(nck, 512)],
                                     qpeT[:, m, ts(nck, 512)], bcs_q[nck][:])
        # kv: scale kvnT in place (runs on DVE under cover of the q_b chains)
        for m in range(KT_KVR):
            for nck in range(2):
                nc.vector.tensor_mul(kvnT[:, m, ts(nck, 512)],
                                     kvnT[:, m, ts(nck, 512)], bcs_k[nck][:])

        # ---- gate logits: rows [2, S] via M=2/N=512 matmuls + exp. Emitted
        # here so the scalar/DVE softmax latency hides under the knope/v_s
        # chains; the tiny finish matmuls run after v_s.
        ers = []
        for nck in range(2):
            psg_t = pt.tile([128, 512], F32, tag="pt")
            psg = psg_t[0:2, :]
            for k in range(KT_HID):
                nc.tensor.matmul(psg, lhsT=gwt[:, k, :],
                                 rhs=xk(k)[:, ts(nck, 512)],
                                 start=(k == 0), stop=(k == KT_HID - 1))
            er = wk1.tile([2, 512], BF, tag="er", bufs=2, name=f"er{nck}")
            nc.scalar.activation(er[:], psg, Exp, bias=gbias[:, 0:1])
            ers.append(er)

        for h in range(HPC):
            for nck in range(2):
                ps = pt.tile([128, 512], F32, tag="pt")
                for k in range(KT_KVR):
                    nc.tensor.matmul(ps[:], lhsT=wkbn[:, k, ts(h, 128)],
                                     rhs=kvnT[:, k, ts(nck, 512)],
                                     start=(k == 0), stop=(k == KT_KVR - 1))
                nc.any.tensor_copy(knopeT[:, h, ts(nck, 512)], ps[:])

        # gate softmax rows (no max-subtract; |logit| is O(1)): the DVE work
        # drains while PE runs the v_s chains below
        grows = []
        for nck in range(2):
            er = ers[nck]
            psd_t = pt.tile([128, 512], F32, tag="pt")
            nc.tensor.matmul(psd_t[0:1, :], lhsT=ones_col[0:2, :], rhs=er[:],
                             start=True, stop=True)
            pse_t = pt.tile([128, 512], F32, tag="pt")
            nc.tensor.matmul(pse_t[0:1, :], lhsT=ident2[:, 1:2], rhs=er[:],
                             start=True, stop=True)
            dinv = wk1.tile([1, 512], F32, tag="dinv", bufs=1)
            nc.vector.reciprocal_approx_fast(out=dinv[:], in_=psd_t[0:1, :])
            g0row = wk1.tile([1, 512], BF, tag="g0row", bufs=2,
                             name=f"g0row{nck}")
            nc.vector.tensor_mul(g0row[:], er[0:1, :], dinv[:])
            g1row = wk1.tile([1, 512], BF, tag="g1row", bufs=2,
                             name=f"g1row{nck}")
            nc.vector.tensor_mul(g1row[:], pse_t[0:1, :], dinv[:])
            grows.append((g0row, g1row))

        for tb in range(TB):
            ps = pt.tile([128, 512], F32, tag="pt")
            for k in range(KT_KVR):
                nc.tensor.matmul(ps[:], lhsT=kvnT[:, k, ts(tb, 128)],
                                 rhs=wkbv[:, k, :],
                                 start=(k == 0), stop=(k == KT_KVR - 1))
            nc.any.tensor_copy(v_s[:, tb, :], ps[:])

        # ---- gate broadcast: rows were computed under the knope chains, so
        # these four tiny matmuls issue without PE stalls.
        for nck in range(2):
            for row, dst in ((grows[nck][0], g0bc), (grows[nck][1], g1bc)):
                psb = pt.tile([128, 512], F32, tag="pt")
                nc.tensor.matmul(psb[:], lhsT=ones_row[:], rhs=row[:],
                                 start=True, stop=True)
                nc.any.tensor_copy(dst[:, ts(nck, 512)], psb[:])

        es12.close()

        # k-major attention: scoresT[k,q] on PE, unnormalized exp, v-stationary
        # ctx matmuls at N=512, denominators via DVE tree-sum + GpSimd
        # partition all-reduce. `fill` emits PE filler work between the score
        # and ctx matmuls so PE never waits on the softmax chain.
        def attention(h, qh, qnT, knT, qpT, kpT, vv, voff, ctxT, is_main, awk,
                      gbc, scp, ctp, fill=None):
            probsT = awk.tile([128, TB, 512], BF, tag="probsT", bufs=3)
            for kb in range(TB):
                ps = scp[0].tile([128, 512], F32, tag=scp[1])
                nc.tensor.matmul(ps[:], lhsT=knT[:, h, ts(kb, 128)],
                                 rhs=qnT[:, h, ts(qh, 512)],
                                 start=True, stop=not is_main)
                if is_main:
                    pb = (h % 2) * 64
                    nc.tensor.matmul(
                        ps[:],
                        lhsT=kpT[pb:pb + 64, h // 2, ts(kb, 128)],
                        rhs=qpT[pb:pb + 64, h // 2, ts(qh, 512)],
                        start=False, stop=True)
                nc.scalar.activation(probsT[:, kb, :], ps[:], Exp)
            if fill is not None:
                fill()
            tr = [awk.tile([128, 512], BF, tag=f"tr{i}", name=f"tr{i}",
                           bufs=(4 if i == 0 else 2)) for i in range(4)]
            for i in range(4):
                nc.vector.tensor_add(tr[i][:], probsT[:, 2 * i, :],
                                     probsT[:, 2 * i + 1, :])
            nc.vector.tensor_add(tr[0][:], tr[0][:], tr[1][:])
            nc.vector.tensor_add(tr[2][:], tr[2][:], tr[3][:])
            nc.vector.tensor_add(tr[0][:], tr[0][:], tr[2][:])
            ct = ctp[0].tile([128, 512], F32, tag=ctp[1])
            for kb in range(TB):
                nc.tensor.matmul(ct[:], lhsT=vv[:, kb, voff:voff + 128],
                                 rhs=probsT[:, kb, :],
                                 start=(kb == 0), stop=(kb == TB - 1))
            ars = awk.tile([128, 512], F32, tag="ars", bufs=3)
            nc.gpsimd.partition_all_reduce(ars[:], tr[0][:], 128,
                                           bass_isa.ReduceOp.add)
            inv = awk.tile([128, 512], F32, tag="inv", bufs=3)
            nc.vector.reciprocal_approx_fast(out=inv[:], in_=ars[:])
            # fold the per-token gate into the softmax normalizer
            invg = awk.tile([128, 512], F32, tag="invg", bufs=3)
            nc.vector.tensor_mul(invg[:], inv[:], gbc[:, ts(qh, 512)])
            nc.vector.tensor_mul(ctxT[:, h, ts(qh, 512)], ct[:], invg[:])

        # ---------- Stage 4a: MLA attention + fillers ----------
        # filler queue: pattern projections (24 chains) + gate (8 tb pieces)
        with tc.tile_pool(name="pf", bufs=2, space="PSUM") as pf:
            patt_work = ([("pq", m, nck) for m in range(HPC) for nck in range(2)]
                         + [("pk", m, nck) for m in range(HPC) for nck in range(2)]
                         + [("pv", tb, 0) for tb in range(TB)])
            copy_flip = [0]

            def emit_patt(n):
                for _ in range(n):
                    if not patt_work:
                        return
                    kind, a, nck = patt_work.pop(0)
                    ps = pf.tile([128, 512], F32, tag="pf")
                    if kind == "pq" or kind == "pk":
                        w, dst = (wspq, pqT) if kind == "pq" else (wspk, pkT)
                        for k in range(KT_HID):
                            nc.tensor.matmul(ps[:], lhsT=w[:, k, ts(a, 128)],
                                             rhs=xk(k)[:, ts(nck, 512)],
                                             start=(k == 0), stop=(k == KT_HID - 1))
                        dslice = dst[:, a, ts(nck, 512)]
                    else:
                        for k in range(KT_HID):
                            nc.tensor.matmul(ps[:], lhsT=xk(k)[:, ts(a, 128)],
                                             rhs=wspv[:, k, :],
                                             start=(k == 0), stop=(k == KT_HID - 1))
                        dslice = pv_s[:, a, :]
                    if copy_flip[0] % 3 < 2:
                        nc.scalar.activation(dslice, ps[:], Copy)
                    else:
                        nc.vector.tensor_copy(dslice, ps[:])
                    copy_flip[0] += 1

            with tc.tile_pool(name="awk", bufs=2) as awk:
                def mla_fill():
                    emit_patt(3)

                for h in range(HPC):
                    for qh in range(2):
                        attention(h, qh, qnopeT, knopeT, qpeT, kpeT,
                                  v_s, h * D_V, ctxT_m, True, awk, g0bc,
                                  (pp, "pp"), (pt, "pt"), fill=mla_fill)
                emit_patt(len(patt_work))

        es_ain.close()
        es_xp.close()
        es_pp.close()

        # ---------- Stage 4b: pattern attention + o-proj/stage5 fillers ----
        with (
            tc.tile_pool(name="pmp", bufs=1) as pmp,
            tc.tile_pool(name="ow", bufs=2) as ow,
            tc.tile_pool(name="ctxp2", bufs=1) as ctxp2,
            tc.tile_pool(name="sc2", bufs=3, space="PSUM") as sc2,
            tc.tile_pool(name="ct2", bufs=2, space="PSUM") as ct2,
            tc.tile_pool(name="po", bufs=3, space="PSUM") as po,
        ):
            ctxT_p = ctxp2.tile([128, HPC, S], BF, tag="ctxp")
            pm_sbs = {}
            pm_work = [(tb, ck) for tb in range(TB) for ck in range(4)]

            def emit_pm(n):
                for _ in range(n):
                    if not pm_work:
                        return
                    tb, ck = pm_work.pop(0)
                    pm = po.tile([128, 512], F32, tag="po")
                    for k in range(KT_KVR):
                        nc.tensor.matmul(pm[:], lhsT=ctxT_m[:, k, ts(tb, 128)],
                                         rhs=wo[:, k, ts(ck, 512)],
                                         start=(k == 0), stop=(k == KT_KVR - 1))
                    pm_sb = pmp.tile([128, 512], BF, tag="pmsb", bufs=32,
                                     name=f"pmsb{tb}_{ck}")
                    # g0 is already folded into ctxT_m; plain psum->sbuf move
                    if ck % 2 == 0:
                        nc.scalar.activation(pm_sb[:], pm[:], Copy)
                    else:
                        nc.vector.tensor_copy(pm_sb[:], pm[:])
                    pm_sbs[(tb, ck)] = pm_sb

            def emit_stage5(tb):
                osb = ow.tile([128, HID], BF, tag="osb")
                for ck in range(4):
                    pq2 = po.tile([128, 512], F32, tag="po")
                    for k in range(KT_KVR):
                        nc.tensor.matmul(
                            pq2[:], lhsT=ctxT_p[:, k, ts(tb, 128)],
                            rhs=wspo[:, k, ts(ck, 512)],
                            start=(k == 0), stop=(k == KT_KVR - 1))
                    # g1 already folded into ctxT_p: combine is a plain add
                    nc.vector.tensor_add(osb[:, ts(ck, 512)], pq2[:],
                                         pm_sbs[(tb, ck)][:])
                nc.gpsimd.dma_start(out=out_d[ts(tb, 128), :], in_=osb[:])

            with tc.tile_pool(name="awk2", bufs=2) as awk2:
                it2 = [0]

                def pat_fill():
                    i = it2[0]
                    emit_pm(4)
                    if i >= 4:
                        emit_stage5(i - 4)

                for qh in range(2):
                    for h in range(HPC):
                        attention(h, qh, pqT, pkT, None, None,
                                  pv_s, h * PD, ctxT_p, False, awk2, g1bc,
                                  (sc2, "sc2"), (ct2, "ct2"), fill=pat_fill)
                        it2[0] += 1
                emit_pm(len(pm_work))

            for tb in range(4, TB):
                emit_stage5(tb)

    nc.compile()
    return nc


def _rope_tables():
    inv_freq = 1.0 / (THETA ** (np.arange(0, D_ROPE, 2, dtype=np.float32) / D_ROPE))
    t = np.arange(S, dtype=np.float32)
    freqs = np.outer(t, inv_freq)                       # [S, 32]
    emb = np.concatenate([freqs, freqs], -1)            # [S, 64]
    cosT = np.cos(emb).T.astype(np.float32)             # [64, S]
    sinT = np.sin(emb).T.astype(np.float32)
    cos2T = np.ascontiguousarray(np.concatenate([cosT, cosT], 0))   # [128, S]
    sin2T = np.ascontiguousarray(np.concatenate([sinT, sinT], 0))
    return cos2T.astype(BF16), sin2T.astype(BF16)


def _prep_in_maps(hidden_states, q_a_w, q_a_ln_w, q_b_w, kv_a_w, kv_a_ln_w,
                  kv_b_w, o_w, sp_q_w, sp_k_w, sp_v_w, sp_o_w, gate_w, gate_b):
    def bf(x):
        return np.ascontiguousarray(x).astype(BF16)

    cos2T, sin2T = _rope_tables()
    qa_wT = bf(q_a_w.T)                                   # [HID, QR]
    kvl_wT = bf(kv_a_w[:KVR].T)                           # [HID, KVR]
    kv_a_pe = kv_a_w[KVR:].reshape(H, D_ROPE, HID)        # [H, 64, HID]

    qb = (q_b_w * q_a_ln_w[None, :]).reshape(H, D_Q, QR) * (D_Q ** -0.5)
    qb_nope = qb[:, :D_NOPE]                              # [H,128,QR]
    qb_pe = qb[:, D_NOPE:]                                # [H,64,QR]
    kvb = (kv_b_w * kv_a_ln_w[None, :]).reshape(H, D_NOPE + D_V, KVR)
    kb_nope = kvb[:, :D_NOPE]                             # [H,128,KVR]
    kb_v = kvb[:, D_NOPE:]                                # [H,128,KVR]
    o_wh = o_w.reshape(HID, H, D_V)                       # [HID,H,128]
    spq = (sp_q_w * (PD ** -0.5)).reshape(PH, PD, HID)
    spk = sp_k_w.reshape(PH, PD, HID)
    spv = sp_v_w.reshape(PH, PD, HID)
    spo = sp_o_w.reshape(HID, PH, PD)
    gate_wT = bf(gate_w.T)                                # [HID, 2]
    gate_bias = np.ascontiguousarray(gate_b.reshape(2, 1)).astype(np.float32)

    in_maps = []
    for c in range(NCORES):
        b, g = c // 4, c % 4
        hs = slice(4 * g, 4 * g + 4)
        m = {
            "xT": bf(hidden_states[b].T),
            "qa_wT": qa_wT,
            "qbn_wT": bf(qb_nope[hs].reshape(HPC * D_NOPE, QR).T),
            "qbp_wT": bf(qb_pe[hs].reshape(HPC * D_ROPE, QR).T),
            "kvl_wT": kvl_wT,
            "kvp_wT": bf(kv_a_pe[hs].reshape(HPC * D_ROPE, HID).T),
            "kbn_wT": bf(kb_nope[hs].reshape(HPC * D_NOPE, KVR).T),
            "kbv_wT": bf(kb_v[hs].reshape(HPC * D_V, KVR).T),
            "o_wT": bf(o_wh[:, hs].reshape(HID, HPC * D_V).T),
            "spq_wT": bf(spq[hs].reshape(HPC * PD, HID).T),
            "spk_wT": bf(spk[hs].reshape(HPC * PD, HID).T),
            "spv_wT": bf(spv[hs].reshape(HPC * PD, HID).T),
            "spo_wT": bf(spo[:, hs].reshape(HID, HPC * PD).T),
            "gate_wT": gate_wT,
            "gate_bias": gate_bias,
            "ident2": np.eye(2, dtype=np.float32).astype(BF16),
            "cos2T": cos2T,
            "sin2T": sin2T,
        }
        in_maps.append(m)
    return in_maps


def kernel(**inputs):
    global LAST_RESULT
    from concourse.bass_utils import run_bass_kernel_spmd

    inputs = {k: np.asarray(v) for k, v in inputs.items()}
    if "nc" not in _graph_cache:
        _graph_cache["nc"] = _build_graph()
    nc = _graph_cache["nc"]

    in_maps = _prep_in_maps(**inputs)
    res = run_bass_kernel_spmd(nc, in_maps, core_ids=list(range(NCORES)),
                               trace=TRACE, **RUN_KWARGS)
    LAST_RESULT = res
    out = np.zeros((B, S, HID), np.float32)
    for c in range(NCORES):
        out[c // 4] += res.results[c]["out"]
    return out



# revision 49
# speedup vs baseline: 1.0027x; 1.0027x over previous
"""ARCAttention (MLA + pattern-attention + gate) distributed Bass kernel for 8 TRN2 NeuronCores.

Sharding: data-parallel over batch (B=2) x tensor-parallel over heads (4 head-groups).
Core c handles batch (c // 4), heads [4*(c%4) .. 4*(c%4)+4) of both the MLA path and the
pattern path. The low-rank a-projections (q_a, kv_a lora) and the gate are replicated
within a batch group. Each core emits a partial (already gate-weighted) output
[S, HID]; the host sums the 4 partials per batch. No device collectives.

Scheduling notes (v2):
- startup: x / first q_a weight chunk split into per-4k tiles; the first six q_a
  chains advance k-group-wise in a dedicated 6-bank PSUM pool so PE starts ~3us in.
- rmsnorm: Rsqrt activation directly (no DVE reciprocal); the per-token scale
  commutes through the b-projections, so it is applied post-hoc to qnope/qpe and
  in-place on kvnT under cover of the q_b chains. Nothing serializes on it.
- gate + pattern q/k/v projections are emitted as PE filler work inside the MLA
  attention loop (between scores and ctx matmuls), keeping PE busy while
  scalar/vector/gpsimd chew softmax.
- main-path o-proj runs as filler inside pattern attention with the g0 gate scale
  folded into the psum->sbuf copy; stage-5 combine is one scalar_tensor_tensor per
  chunk and tb 0-3 are emitted inside the pattern loop.
"""

import numpy as np
import ml_dtypes

# ---- model config (hardcoded from the problem spec) ----
B, S, HID = 2, 1024, 2048
H = 16
D_NOPE, D_ROPE, D_V = 128, 64, 128
D_Q = D_NOPE + D_ROPE            # 192
QR, KVR = 1536, 512
PH, PD = 16, 128
THETA, EPS = 10000.0, 1e-6
NCORES = 8
HPC = 4                          # heads per core
TB = S // 128                    # 8 token blocks
KT_HID = HID // 128              # 16
KT_QR = QR // 128                # 12
KT_KVR = KVR // 128              # 4

BF16 = ml_dtypes.bfloat16

# knobs for test harness
TRACE = False
RUN_KWARGS = {}
LAST_RESULT = None

_graph_cache = {}


def _build_graph():
    from contextlib import ExitStack
    import concourse.bass as bass
    import concourse.mybir as mybir
    import concourse.tile as tile
    from concourse import bacc, bass_isa

    BF = mybir.dt.bfloat16
    F32 = mybir.dt.float32
    Exp = mybir.ActivationFunctionType.Exp
    Square = mybir.ActivationFunctionType.Square
    Sqrt = mybir.ActivationFunctionType.Sqrt
    Copy = mybir.ActivationFunctionType.Copy
    MULT = mybir.AluOpType.mult
    ADD = mybir.AluOpType.add
    X = mybir.AxisListType.X
    ts = bass.ts

    nc = bacc.Bacc("TRN2", target_bir_lowering=False, debug=False,
                   num_devices=NCORES)

    def din(name, shape, dt=BF):
        return nc.declare_dram_parameter(name, list(shape), dt, isOutput=False)

    xT_d = din("xT", [HID, S])
    qa_d = din("qa_wT", [HID, QR])
    qbn_d = din("qbn_wT", [QR, HPC * D_NOPE])
    qbp_d = din("qbp_wT", [QR, HPC * D_ROPE])
    kvl_d = din("kvl_wT", [HID, KVR])
    kvp_d = din("kvp_wT", [HID, HPC * D_ROPE])
    kbn_d = din("kbn_wT", [KVR, HPC * D_NOPE])
    kbv_d = din("kbv_wT", [KVR, HPC * D_V])
    ow_d = din("o_wT", [HPC * D_V, HID])
    spq_d = din("spq_wT", [HID, HPC * PD])
    spk_d = din("spk_wT", [HID, HPC * PD])
    spv_d = din("spv_wT", [HID, HPC * PD])
    spo_d = din("spo_wT", [HPC * PD, HID])
    gw_d = din("gate_wT", [HID, 2])
    gb_d = din("gate_bias", [2, 1], F32)
    id2_d = din("ident2", [2, 2])
    cos_d = din("cos2T", [128, S])
    sin_d = din("sin2T", [128, S])
    out_d = nc.declare_dram_parameter("out", [S, HID], BF, isOutput=True)

    def r3(dram, kt):
        # [kt*128, N] dram tensor viewed as [128, kt, N] for SBUF tiling
        return dram.ap().rearrange("(k p) n -> p k n", p=128, k=kt)

    es = ExitStack()
    with tile.TileContext(nc) as tc, es:
        constp = es.enter_context(tc.tile_pool(name="const", bufs=1))
        wring = es.enter_context(tc.tile_pool(name="wring", bufs=2))
        ainp = es.enter_context(tc.tile_pool(name="ain_pat", bufs=1))
        ctxp = es.enter_context(tc.tile_pool(name="ctxp", bufs=1))
        # xp is opened after the es-scoped pools and closed right after stage
        # 4a (pools release LIFO), freeing x's 32KB/partition for stage 4b
        es_xp = ExitStack()
        xp = es_xp.enter_context(tc.tile_pool(name="xp", bufs=1))

        eps_t = constp.tile([128, 1], F32, tag="eps")
        nc.vector.memset(eps_t[:], EPS)
        ones_col = constp.tile([128, 1], BF, tag="ones_col")
        nc.vector.memset(ones_col[:], 1.0)
        ones_row = constp.tile([1, 128], BF, tag="ones_row")
        nc.vector.memset(ones_row[:], 1.0)
        ident2 = constp.tile([2, 2], BF, tag="ident2")
        cosT = constp.tile([128, S], BF, tag="cos")
        sinT = constp.tile([128, S], BF, tag="sin")
        gbias = constp.tile([2, 1], F32, tag="gb")
        gwt = constp.tile([128, KT_HID, 2], BF, tag="gw")

        # x split into 4 k-group tiles for fine-grained startup deps
        xTs = [xp.tile([128, 4, S], BF, tag=f"xT{g}", name=f"xT{g}")
               for g in range(4)]

        def xk(k):
            return xTs[k // 4][:, k % 4, :]

        # pattern outputs (written as fillers during MLA attention)
        pqT = ainp.tile([128, HPC, S], BF, tag="pqT")
        pkT = ainp.tile([128, HPC, S], BF, tag="pkT")
        pv_s = ainp.tile([128, TB, HPC * PD], BF, tag="pv")
        # per-token gates broadcast across partitions; folded into the
        # attention ctx normalization (ctxT_m pre-scaled by g0, ctxT_p by g1)
        g0bc = ainp.tile([128, S], BF, tag="g0bc")
        g1bc = ainp.tile([128, S], BF, tag="g1bc")

        ctxT_m = ctxp.tile([128, HPC, S], BF, tag="ctxm")

        # ---- weight ring: rotating 16KB/partition chunks ----
        def ring_chunk(name):
            return wring.tile([128, 8192], BF, tag="w", name=name)

        def kview(ap, k, n):
            return ap.rearrange("p (k n) -> p k n", k=k, n=n)

        def dma4(view, src_r3):
            for kq in range(0, view.shape[1], 4):
                hi = min(kq + 4, view.shape[1])
                nc.sync.dma_start(out=view[:, kq:hi, :], in_=src_r3[:, kq:hi, :])

        # MLA attention inputs (freed after MLA attention)
        es_ain = ExitStack()
        ain = es_ain.enter_context(tc.tile_pool(name="ain_mla", bufs=1))
        qnopeT = ain.tile([128, HPC, S], BF, tag="qnopeT")
        qpeT = ain.tile([128, 2, S], BF, tag="qpeT")
        knopeT = ain.tile([128, HPC, S], BF, tag="knopeT")
        kpeT = ain.tile([128, 2, S], BF, tag="kpeT")
        v_s = ain.tile([128, TB, HPC * D_V], BF, tag="v")   # token-major

        # stage-1/2 scratch (scoped; freed before attention)
        es12 = ExitStack()
        q2 = es12.enter_context(tc.tile_pool(name="q2", bufs=1))
        wk1 = es12.enter_context(tc.tile_pool(name="wk1", bufs=3))
        wrope = es12.enter_context(tc.tile_pool(name="wrope", bufs=1))
        qmidT = q2.tile([128, KT_QR, S], BF, tag="qmidT")
        kvnT = q2.tile([128, KT_KVR, S], BF, tag="kvnT")
        # shared between the q and k rms phases: emit_bcs(q) fully drains
        # these before the kvl stashes re-fill them
        sqacc = [q2.tile([128, 512], F32, tag=f"sqa{i}", name=f"sqa{i}")
                 for i in range(2)]
        # first q_a weight chunk, split per k-group (freed right after boot)
        es_qa0 = ExitStack()
        qa0p = es_qa0.enter_context(tc.tile_pool(name="qa0p", bufs=1))
        wq0s = [qa0p.tile([128, 4, 512], BF, tag=f"qa0_{g}", name=f"qa0_{g}")
                for g in range(4)]
        bcs_q = [q2.tile([128, 512], BF, tag=f"bcsq{i}", name=f"bcsq{i}")
                 for i in range(2)]
        bcs_k = [q2.tile([128, 512], BF, tag=f"bcsk{i}", name=f"bcsk{i}")
                 for i in range(2)]

        # startup DMAs: x spread across four engine queues (per-k slices for
        # the first k-group) so the HBM bandwidth of several DMA queues feeds
        # the boot chains; wq0 chunks go first on the sync queue.
        nc.sync.dma_start(out=wq0s[0][:], in_=r3(qa_d, KT_HID)[:, 0:4, ts(0, 512)])
        for j in range(2):
            nc.scalar.dma_start(out=xTs[0][:, j, :], in_=r3(xT_d, KT_HID)[:, j, :])
        for j in range(2, 4):
            nc.sync.dma_start(out=xTs[0][:, j, :], in_=r3(xT_d, KT_HID)[:, j, :])
        nc.gpsimd.dma_start(out=xTs[1][:], in_=r3(xT_d, KT_HID)[:, 4:8, :])
        nc.scalar.dma_start(out=xTs[2][:], in_=r3(xT_d, KT_HID)[:, 8:12, :])
        nc.gpsimd.dma_start(out=xTs[3][:], in_=r3(xT_d, KT_HID)[:, 12:16, :])
        for g in range(1, 4):
            nc.sync.dma_start(out=wq0s[g][:],
                              in_=r3(qa_d, KT_HID)[:, 4 * g:4 * g + 4, ts(0, 512)])
        wt_qa = [None]
        for ck in (1, 2):
            chq = ring_chunk(f"qa{ck}")
            v = kview(chq, KT_HID, 512)
            dma4(v, r3(qa_d, KT_HID)[:, :, ts(ck, 512)].rearrange("p k n -> p k n"))
            wt_qa.append(v)
        ch_kvl = ring_chunk("kvl")
        wt_kl = kview(ch_kvl, KT_HID, 512)
        dma4(wt_kl, r3(kvl_d, KT_HID))
        ch_kvp = ring_chunk("kvp")
        wt_kp = kview(ch_kvp[:, 0:4096], KT_HID, HPC * D_ROPE)
        nc.sync.dma_start(out=wt_kp[:], in_=r3(kvp_d, KT_HID))
        nc.sync.dma_start(out=gwt[:], in_=r3(gw_d, KT_HID))
        nc.sync.dma_start(out=gbias[:], in_=gb_d.ap())
        nc.sync.dma_start(out=ident2[:], in_=id2_d.ap())
        nc.sync.dma_start(out=cosT[:], in_=cos_d.ap())
        nc.sync.dma_start(out=sinT[:], in_=sin_d.ap())
        ch_w2a = ring_chunk("w2a")
        wqbn = kview(ch_w2a[:, 0:6144], KT_QR, HPC * D_NOPE)
        nc.sync.dma_start(out=wqbn[:], in_=r3(qbn_d, KT_QR))
        wkbn = kview(ch_w2a[:, 6144:8192], KT_KVR, HPC * D_NOPE)
        nc.sync.dma_start(out=wkbn[:], in_=r3(kbn_d, KT_KVR))
        ch_w2b = ring_chunk("w2b")
        wqbp = kview(ch_w2b[:, 0:3072], KT_QR, HPC * D_ROPE)
        nc.sync.dma_start(out=wqbp[:], in_=r3(qbp_d, KT_QR))
        wkbv = kview(ch_w2b[:, 3072:5120], KT_KVR, HPC * D_V)
        nc.sync.dma_start(out=wkbv[:], in_=r3(kbv_d, KT_KVR))
        ch_spq = ring_chunk("spq")
        wspq = kview(ch_spq, KT_HID, HPC * PD)
        dma4(wspq, r3(spq_d, KT_HID))
        ch_spk = ring_chunk("spk")
        wspk = kview(ch_spk, KT_HID, HPC * PD)
        dma4(wspk, r3(spk_d, KT_HID))
        ch_spv = ring_chunk("spv")
        wspv = kview(ch_spv, KT_HID, HPC * PD)
        dma4(wspv, r3(spv_d, KT_HID))
        ch_wo = ring_chunk("wo")
        wo = kview(ch_wo, KT_KVR, HID)
        nc.sync.dma_start(out=wo[:], in_=r3(ow_d, KT_KVR))
        ch_wspo = ring_chunk("wspo")
        wspo = kview(ch_wspo, KT_KVR, HID)
        nc.sync.dma_start(out=wspo[:], in_=r3(spo_d, KT_KVR))

        # sum-of-squares: square on scalar (psum read), accumulate on the
        # otherwise-idle gpsimd engine into per-nck f32 tiles. No PE
        # ones-matmuls, no psum bank for ssq.
        sq_seen = set()

        def stash_sq(ps, nck, phase):
            sq = wk1.tile([128, 512], BF, tag="sq", bufs=5)
            nc.scalar.activation(sq[:], ps[:], Square)
            skey = (phase, nck)
            if skey not in sq_seen:
                sq_seen.add(skey)
                nc.vector.tensor_copy(sqacc[nck][:], sq[:])
            else:
                nc.vector.tensor_add(sqacc[nck][:], sqacc[nck][:], sq[:])

        # rms scale factors: rsqrt(ssq/n + eps) as [128,512] bf16. The
        # partition reduction runs on gpsimd (broadcast output for free),
        # sqrt on scalar, and an approx-fast reciprocal on DVE.
        def emit_bcs(n, dst2):
            for nck in range(2):
                arsb = wrope.tile([128, 512], F32, tag="rot", bufs=1)
                nc.gpsimd.partition_all_reduce(arsb[:], sqacc[nck][:], 128,
                                               bass_isa.ReduceOp.add)
                nc.scalar.activation(sqacc[nck][:], arsb[:], Sqrt,
                                     bias=eps_t[:, 0:1], scale=1.0 / n)
                nc.vector.reciprocal_approx_fast(out=arsb[:], in_=sqacc[nck][:])
                nc.vector.tensor_copy(dst2[nck][:], arsb[:])

        # ================= stage 1 =================
        # boot: first 6 q_a chains (m0-2 x nck0-1) advance k-group-wise so PE
        # starts as soon as the first x / weight k-groups land.
        with tc.tile_pool(name="boot", bufs=6, space="PSUM") as bootp:
            chains6 = [(m, nck) for m in range(3) for nck in range(2)]
            bps = [bootp.tile([128, 512], F32, tag="bp", name=f"bp{i}")
                   for i in range(6)]
            for g in range(4):
                for j in range(4):
                    k = 4 * g + j
                    for ci, (m, nck) in enumerate(chains6):
                        nc.tensor.matmul(bps[ci][:],
                                         lhsT=wq0s[g][:, j, ts(m, 128)],
                                         rhs=xTs[g][:, j, ts(nck, 512)],
                                         start=(k == 0), stop=(k == KT_HID - 1))
            for ci, (m, nck) in enumerate(chains6):
                nc.any.tensor_copy(qmidT[:, m, ts(nck, 512)], bps[ci][:])
                stash_sq(bps[ci], nck, 'q')
            # m3 chains reuse boot slots
            for nck in range(2):
                bp = bootp.tile([128, 512], F32, tag="bp", name=f"bp6_{nck}")
                for g in range(4):
                    for j in range(4):
                        k = 4 * g + j
                        nc.tensor.matmul(bp[:], lhsT=wq0s[g][:, j, ts(3, 128)],
                                         rhs=xTs[g][:, j, ts(nck, 512)],
                                         start=(k == 0), stop=(k == KT_HID - 1))
                nc.any.tensor_copy(qmidT[:, 3, ts(nck, 512)], bp[:])
                stash_sq(bp, nck, 'q')
        es_qa0.close()

        es_pp = ExitStack()
        pp = es_pp.enter_context(tc.tile_pool(name="pp", bufs=3, space="PSUM"))
        pt = es_pp.enter_context(tc.tile_pool(name="pt", bufs=3, space="PSUM"))

        # q_a chunks 1,2
        for ck in (1, 2):
            wt = wt_qa[ck]
            for mm4 in range(4):
                m = ck * 4 + mm4
                for nck in range(2):
                    ps = pp.tile([128, 512], F32, tag="pp")
                    for k in range(KT_HID):
                        nc.tensor.matmul(ps[:], lhsT=wt[:, k, ts(mm4, 128)],
                                         rhs=xk(k)[:, ts(nck, 512)],
                                         start=(k == 0), stop=(k == KT_HID - 1))
                    nc.any.tensor_copy(qmidT[:, m, ts(nck, 512)], ps[:])
                    stash_sq(ps, nck, 'q')

        emit_bcs(QR, bcs_q)

        # kv_a lora part: feature-major [KVR, S]
        for m in range(KT_KVR):
            for nck in range(2):
                ps = pp.tile([128, 512], F32, tag="pp")
                for k in range(KT_HID):
                    nc.tensor.matmul(ps[:], lhsT=wt_kl[:, k, ts(m, 128)],
                                     rhs=xk(k)[:, ts(nck, 512)],
                                     start=(k == 0), stop=(k == KT_HID - 1))
                nc.any.tensor_copy(kvnT[:, m, ts(nck, 512)], ps[:])
                stash_sq(ps, nck, 'k')

        
        def rope_from_psum(ps, dst, nck, work):
            """Apply rope to a [128, 512] psum chunk holding 2 stacked
            64-dim pe heads; write bf16 to dst ([128,512] slice)."""
            rot = work.tile([128, 512], F32, tag="rot")
            nc.vector.tensor_scalar_mul(rot[0:32, :], ps[32:64, :], -1.0)
            nc.vector.tensor_copy(rot[32:64, :], ps[0:32, :])
            nc.vector.tensor_scalar_mul(rot[64:96, :], ps[96:128, :], -1.0)
            nc.vector.tensor_copy(rot[96:128, :], ps[64:96, :])
            nc.vector.tensor_mul(dst, ps[:], cosT[:, ts(nck, 512)])
            nc.vector.tensor_mul(rot[:], rot[:], sinT[:, ts(nck, 512)])
            nc.vector.tensor_add(dst, dst, rot[:])

        # kv_a pe part (2 m-tiles of 2 stacked heads) + rope
        for m in range(2):
            for nck in range(2):
                ps = pt.tile([128, 512], F32, tag="pt")
                for k in range(KT_HID):
                    nc.tensor.matmul(ps[:], lhsT=wt_kp[:, k, ts(m, 128)],
                                     rhs=xk(k)[:, ts(nck, 512)],
                                     start=(k == 0), stop=(k == KT_HID - 1))
                rope_from_psum(ps, kpeT[:, m, ts(nck, 512)], nck, wrope)

        emit_bcs(KVR, bcs_k)

        # ---------- Stage 2: b-projections ----------
        # q_b on RAW qmid; the rms scale is applied post-hoc to qnope/qpe
        # (it commutes through the contraction and rope).
        for h in range(HPC):
            for nck in range(2):
                ps = pt.tile([128, 512], F32, tag="pt")
                for k in range(KT_QR):
                    nc.tensor.matmul(ps[:], lhsT=wqbn[:, k, ts(h, 128)],
                                     rhs=qmidT[:, k, ts(nck, 512)],
                                     start=(k == 0), stop=(k == KT_QR - 1))
                nc.any.tensor_copy(qnopeT[:, h, ts(nck, 512)], ps[:])
                nc.vector.tensor_mul(qnopeT[:, h, ts(nck, 512)],
                                     qnopeT[:, h, ts(nck, 512)], bcs_q[nck][:])
        for m in range(2):
            for nck in range(2):
                ps = pt.tile([128, 512], F32, tag="pt")
                for k in range(KT_QR):
                    nc.tensor.matmul(ps[:], lhsT=wqbp[:, k, ts(m, 128)],
                                     rhs=qmidT[:, k, ts(nck, 512)],
                                     start=(k == 0), stop=(k == KT_QR - 1))
                rope_from_psum(ps, qpeT[:, m, ts(nck, 512)], nck, wrope)
                nc.vector.tensor_mul(qpeT[:, m, ts(nck, 512)],
                                     qpeT[:, m, ts(nck, 512)], bcs_q[nck][:])
        # kv: scale kvnT in place (runs on DVE under cover of the q_b chains)
        for m in range(KT_KVR):
            for nck in range(2):
                nc.vector.tensor_mul(kvnT[:, m, ts(nck, 512)],
                                     kvnT[:, m, ts(nck, 512)], bcs_k[nck][:])

        # ---- gate logits: rows [2, S] via M=2/N=512 matmuls + exp. Emitted
        # here so the scalar/DVE softmax latency hides under the knope/v_s
        # chains; the tiny finish matmuls run after v_s.
        ers = []
        for nck in range(2):
            psg_t = pt.tile([128, 512], F32, tag="pt")
            psg = psg_t[0:2, :]
            for k in range(KT_HID):
                nc.tensor.matmul(psg, lhsT=gwt[:, k, :],
                                 rhs=xk(k)[:, ts(nck, 512)],
                                 start=(k == 0), stop=(k == KT_HID - 1))
            er = wk1.tile([2, 512], BF, tag="er", bufs=2, name=f"er{nck}")
            nc.scalar.activation(er[:], psg, Exp, bias=gbias[:, 0:1])
            ers.append(er)

        for h in range(HPC):
            for nck in range(2):
                ps = pt.tile([128, 512], F32, tag="pt")
                for k in range(KT_KVR):
                    nc.tensor.matmul(ps[:], lhsT=wkbn[:, k, ts(h, 128)],
                                     rhs=kvnT[:, k, ts(nck, 512)],
                                     start=(k == 0), stop=(k == KT_KVR - 1))
                nc.any.tensor_copy(knopeT[:, h, ts(nck, 512)], ps[:])

        # gate softmax rows (no max-subtract; |logit| is O(1)): the DVE work
        # drains while PE runs the v_s chains below
        grows = []
        for nck in range(2):
            er = ers[nck]
            psd_t = pt.tile([128, 512], F32, tag="pt")
            nc.tensor.matmul(psd_t[0:1, :], lhsT=ones_col[0:2, :], rhs=er[:],
                             start=True, stop=True)
            pse_t = pt.tile([128, 512], F32, tag="pt")
            nc.tensor.matmul(pse_t[0:1, :], lhsT=ident2[:, 1:2], rhs=er[:],
                             start=True, stop=True)
            dinv = wk1.tile([1, 512], F32, tag="dinv", bufs=1)
            nc.vector.reciprocal_approx_fast(out=dinv[:], in_=psd_t[0:1, :])
            g0row = wk1.tile([1, 512], BF, tag="g0row", bufs=2,
                             name=f"g0row{nck}")
            nc.vector.tensor_mul(g0row[:], er[0:1, :], dinv[:])
            g1row = wk1.tile([1, 512], BF, tag="g1row", bufs=2,
                             name=f"g1row{nck}")
            nc.vector.tensor_mul(g1row[:], pse_t[0:1, :], dinv[:])
            grows.append((g0row, g1row))

        for tb in range(TB):
            ps = pt.tile([128, 512], F32, tag="pt")
            for k in range(KT_KVR):
                nc.tensor.matmul(ps[:], lhsT=kvnT[:, k, ts(tb, 128)],
                                 rhs=wkbv[:, k, :],
                                 start=(k == 0), stop=(k == KT_KVR - 1))
            nc.any.tensor_copy(v_s[:, tb, :], ps[:])

        # ---- gate broadcast: rows were computed under the knope chains, so
        # these four tiny matmuls issue without PE stalls.
        for nck in range(2):
            for row, dst in ((grows[nck][0], g0bc), (grows[nck][1], g1bc)):
                psb = pt.tile([128, 512], F32, tag="pt")
                nc.tensor.matmul(psb[:], lhsT=ones_row[:], rhs=row[:],
                                 start=True, stop=True)
                nc.any.tensor_copy(dst[:, ts(nck, 512)], psb[:])

        es12.close()

        # k-major attention: scoresT[k,q] on PE, unnormalized exp, v-stationary
        # ctx matmuls at N=512, denominators via DVE tree-sum + GpSimd
        # partition all-reduce. `fill` emits PE filler work between the score
        # and ctx matmuls so PE never waits on the softmax chain.
        def attention(h, qh, qnT, knT, qpT, kpT, vv, voff, ctxT, is_main, awk,
                      gbc, scp, ctp, fill=None):
            probsT = awk.tile([128, TB, 512], BF, tag="probsT", bufs=3)
            for kb in range(TB):
                ps = scp[0].tile([128, 512], F32, tag=scp[1])
                nc.tensor.matmul(ps[:], lhsT=knT[:, h, ts(kb, 128)],
                                 rhs=qnT[:, h, ts(qh, 512)],
                                 start=True, stop=not is_main)
                if is_main:
                    pb = (h % 2) * 64
                    nc.tensor.matmul(
                        ps[:],
                        lhsT=kpT[pb:pb + 64, h // 2, ts(kb, 128)],
                        rhs=qpT[pb:pb + 64, h // 2, ts(qh, 512)],
                        start=False, stop=True)
                nc.scalar.activation(probsT[:, kb, :], ps[:], Exp)
            if fill is not None:
                fill()
            tr = [awk.tile([128, 512], BF, tag=f"tr{i}", name=f"tr{i}",
                           bufs=(4 if i == 0 else 2)) for i in range(4)]
            for i in range(4):
                nc.vector.tensor_add(tr[i][:], probsT[:, 2 * i, :],
                                     probsT[:, 2 * i + 1, :])
            nc.vector.tensor_add(tr[0][:], tr[0][:], tr[1][:])
            nc.vector.tensor_add(tr[2][:], tr[2][:], tr[3][:])
            nc.vector.tensor_add(tr[0][:], tr[0][:], tr[2][:])
            ct = ctp[0].tile([128, 512], F32, tag=ctp[1])
            for kb in range(TB):
                nc.tensor.matmul(ct[:], lhsT=vv[:, kb, voff:voff + 128],
                                 rhs=probsT[:, kb, :],
                                 start=(kb == 0), stop=(kb == TB - 1))
            ars = awk.tile([128, 512], F32, tag="ars", bufs=3)
            nc.gpsimd.partition_all_reduce(ars[:], tr[0][:], 128,
                                           bass_isa.ReduceOp.add)
            inv = awk.tile([128, 512], F32, tag="inv", bufs=3)
            nc.vector.reciprocal_approx_fast(out=inv[:], in_=ars[:])
            # fold the per-token gate into the softmax normalizer
            invg = awk.tile([128, 512], F32, tag="invg", bufs=3)
            nc.vector.tensor_mul(invg[:], inv[:], gbc[:, ts(qh, 512)])
            nc.vector.tensor_mul(ctxT[:, h, ts(qh, 512)], ct[:], invg[:])

        # ---------- Stage 4a: MLA attention + fillers ----------
        # filler queue: pattern projections (24 chains) + gate (8 tb pieces)
        with tc.tile_pool(name="pf", bufs=2, space="PSUM") as pf:
            patt_work = ([("pq", m, nck) for m in range(HPC) for nck in range(2)]
                         + [("pk", m, nck) for m in range(HPC) for nck in range(2)])
            copy_flip = [0]

            def emit_patt(n):
                for _ in range(n):
                    if not patt_work:
                        return
                    kind, a, nck = patt_work.pop(0)
                    ps = pf.tile([128, 512], F32, tag="pf")
                    if kind == "pq" or kind == "pk":
                        w, dst = (wspq, pqT) if kind == "pq" else (wspk, pkT)
                        for k in range(KT_HID):
                            nc.tensor.matmul(ps[:], lhsT=w[:, k, ts(a, 128)],
                                             rhs=xk(k)[:, ts(nck, 512)],
                                             start=(k == 0), stop=(k == KT_HID - 1))
                        dslice = dst[:, a, ts(nck, 512)]
                    else:
                        for k in range(KT_HID):
                            nc.tensor.matmul(ps[:], lhsT=xk(k)[:, ts(a, 128)],
                                             rhs=wspv[:, k, :],
                                             start=(k == 0), stop=(k == KT_HID - 1))
                        dslice = pv_s[:, a, :]
                    if copy_flip[0] % 3 < 2:
                        nc.scalar.activation(dslice, ps[:], Copy)
                    else:
                        nc.vector.tensor_copy(dslice, ps[:])
                    copy_flip[0] += 1

            with tc.tile_pool(name="awk", bufs=2) as awk:
                def mla_fill():
                    emit_patt(2)

                for h in range(HPC):
                    for qh in range(2):
                        attention(h, qh, qnopeT, knopeT, qpeT, kpeT,
                                  v_s, h * D_V, ctxT_m, True, awk, g0bc,
                                  (pp, "pp"), (pt, "pt"), fill=mla_fill)
                emit_patt(len(patt_work))

        es_ain.close()
        es_pp.close()

        # ---------- Stage 4b: pattern attention + o-proj/stage5 fillers ----
        with (
            tc.tile_pool(name="pmp", bufs=1) as pmp,
            tc.tile_pool(name="ow", bufs=2) as ow,
            tc.tile_pool(name="ctxp2", bufs=1) as ctxp2,
            tc.tile_pool(name="sc2", bufs=3, space="PSUM") as sc2,
            tc.tile_pool(name="ct2", bufs=2, space="PSUM") as ct2,
            tc.tile_pool(name="po", bufs=3, space="PSUM") as po,
        ):
            ctxT_p = ctxp2.tile([128, HPC, S], BF, tag="ctxp")
            pm_sbs = {}
            pm_work = [(tb, ck) for tb in range(TB) for ck in range(4)]

            def emit_pm(n):
                for _ in range(n):
                    if not pm_work:
                        return
                    tb, ck = pm_work.pop(0)
                    pm = po.tile([128, 512], F32, tag="po")
                    for k in range(KT_KVR):
                        nc.tensor.matmul(pm[:], lhsT=ctxT_m[:, k, ts(tb, 128)],
                                         rhs=wo[:, k, ts(ck, 512)],
                                         start=(k == 0), stop=(k == KT_KVR - 1))
                    pm_sb = pmp.tile([128, 512], BF, tag="pmsb", bufs=32,
                                     name=f"pmsb{tb}_{ck}")
                    # g0 is already folded into ctxT_m; plain psum->sbuf move
                    if ck % 2 == 0:
                        nc.scalar.activation(pm_sb[:], pm[:], Copy)
                    else:
                        nc.vector.tensor_copy(pm_sb[:], pm[:])
                    pm_sbs[(tb, ck)] = pm_sb

            def emit_stage5(tb):
                osb = ow.tile([128, HID], BF, tag="osb")
                for ck in range(4):
                    pq2 = po.tile([128, 512], F32, tag="po")
                    for k in range(KT_KVR):
                        nc.tensor.matmul(
                            pq2[:], lhsT=ctxT_p[:, k, ts(tb, 128)],
                            rhs=wspo[:, k, ts(ck, 512)],
                            start=(k == 0), stop=(k == KT_KVR - 1))
                    # g1 already folded into ctxT_p: combine is a plain add
                    nc.vector.tensor_add(osb[:, ts(ck, 512)], pq2[:],
                                         pm_sbs[(tb, ck)][:])
                nc.gpsimd.dma_start(out=out_d[ts(tb, 128), :], in_=osb[:])

            pv_flip = [0]

            def emit_pv(tb):
                ps = po.tile([128, 512], F32, tag="po")
                for k in range(KT_HID):
                    nc.tensor.matmul(ps[:], lhsT=xk(k)[:, ts(tb, 128)],
                                     rhs=wspv[:, k, :],
                                     start=(k == 0), stop=(k == KT_HID - 1))
                if pv_flip[0] % 2 == 0:
                    nc.scalar.activation(pv_s[:, tb, :], ps[:], Copy)
                else:
                    nc.vector.tensor_copy(pv_s[:, tb, :], ps[:])
                pv_flip[0] += 1

            with tc.tile_pool(name="awk2", bufs=2) as awk2:
                it2 = [0]

                def pat_fill():
                    i = it2[0]
                    if i == 0:
                        # pv chains: only needed by ctx, so they slot between
                        # the first call's scores and ctx as bulk PE fill
                        for tb in range(TB):
                            emit_pv(tb)
                        return
                    emit_pm(5)
                    if i >= 4:
                        emit_stage5(i - 4)

                for qh in range(2):
                    for h in range(HPC):
                        attention(h, qh, pqT, pkT, None, None,
                                  pv_s, h * PD, ctxT_p, False, awk2, g1bc,
                                  (sc2, "sc2"), (ct2, "ct2"), fill=pat_fill)
                        it2[0] += 1
                emit_pm(len(pm_work))

            for tb in range(4, TB):
                emit_stage5(tb)

        es_xp.close()

    nc.compile()
    return nc


def _rope_tables():
    inv_freq = 1.0 / (THETA ** (np.arange(0, D_ROPE, 2, dtype=np.float32) / D_ROPE))
    t = np.arange(S, dtype=np.float32)
    freqs = np.outer(t, inv_freq)                       # [S, 32]
    emb = np.concatenate([freqs, freqs], -1)            # [S, 64]
    cosT = np.cos(emb).T.astype(np.float32)             # [64, S]
    sinT = np.sin(emb).T.astype(np.float32)
    cos2T = np.ascontiguousarray(np.concatenate([cosT, cosT], 0))   # [128, S]
    sin2T = np.ascontiguousarray(np.concatenate([sinT, sinT], 0))
    return cos2T.astype(BF16), sin2T.astype(BF16)


def _prep_in_maps(hidden_states, q_a_w, q_a_ln_w, q_b_w, kv_a_w, kv_a_ln_w,
                  kv_b_w, o_w, sp_q_w, sp_k_w, sp_v_w, sp_o_w, gate_w, gate_b):
    def bf(x):
        return np.ascontiguousarray(x).astype(BF16)

    cos2T, sin2T = _rope_tables()
    qa_wT = bf(q_a_w.T)                                   # [HID, QR]
    kvl_wT = bf(kv_a_w[:KVR].T)                           # [HID, KVR]
    kv_a_pe = kv_a_w[KVR:].reshape(H, D_ROPE, HID)        # [H, 64, HID]

    qb = (q_b_w * q_a_ln_w[None, :]).reshape(H, D_Q, QR) * (D_Q ** -0.5)
    qb_nope = qb[:, :D_NOPE]                              # [H,128,QR]
    qb_pe = qb[:, D_NOPE:]                                # [H,64,QR]
    kvb = (kv_b_w * kv_a_ln_w[None, :]).reshape(H, D_NOPE + D_V, KVR)
    kb_nope = kvb[:, :D_NOPE]                             # [H,128,KVR]
    kb_v = kvb[:, D_NOPE:]                                # [H,128,KVR]
    o_wh = o_w.reshape(HID, H, D_V)                       # [HID,H,128]
    spq = (sp_q_w * (PD ** -0.5)).reshape(PH, PD, HID)
    spk = sp_k_w.reshape(PH, PD, HID)
    spv = sp_v_w.reshape(PH, PD, HID)
    spo = sp_o_w.reshape(HID, PH, PD)
    gate_wT = bf(gate_w.T)                                # [HID, 2]
    gate_bias = np.ascontiguousarray(gate_b.reshape(2, 1)).astype(np.float32)

    in_maps = []
    for c in range(NCORES):
        b, g = c // 4, c % 4
        hs = slice(4 * g, 4 * g + 4)
        m = {
            "xT": bf(hidden_states[b].T),
            "qa_wT": qa_wT,
            "qbn_wT": bf(qb_nope[hs].reshape(HPC * D_NOPE, QR).T),
            "qbp_wT": bf(qb_pe[hs].reshape(HPC * D_ROPE, QR).T),
            "kvl_wT": kvl_wT,
            "kvp_wT": bf(kv_a_pe[hs].reshape(HPC * D_ROPE, HID).T),
            "kbn_wT": bf(kb_nope[hs].reshape(HPC * D_NOPE, KVR).T),
            "kbv_wT": bf(kb_v[hs].reshape(HPC * D_V, KVR).T),
            "o_wT": bf(o_wh[:, hs].reshape(HID, HPC * D_V).T),
            "spq_wT": bf(spq[hs].reshape(HPC * PD, HID).T),
            "spk_wT": bf(spk[hs].reshape(HPC * PD, HID).T),
            "spv_wT": bf(spv[hs].reshape(HPC * PD, HID).T),
            "spo_wT": bf(spo[:, hs].reshape(HID, HPC * PD).T),
            "gate_wT": gate_wT,
            "gate_bias": gate_bias,
            "ident2": np.eye(2, dtype=np.float32).astype(BF16),
            "cos2T": cos2T,
            "sin2T": sin2T,
        }
        in_maps.append(m)
    return in_maps


def kernel(**inputs):
    global LAST_RESULT
    from concourse.bass_utils import run_bass_kernel_spmd

    inputs = {k: np.asarray(v) for k, v in inputs.items()}
    if "nc" not in _graph_cache:
        _graph_cache["nc"] = _build_graph()
    nc = _graph_cache["nc"]

    in_maps = _prep_in_maps(**inputs)
    res = run_bass_kernel_spmd(nc, in_maps, core_ids=list(range(NCORES)),
                               trace=TRACE, **RUN_KWARGS)
    LAST_RESULT = res
    out = np.zeros((B, S, HID), np.float32)
    for c in range(NCORES):
        out[c // 4] += res.results[c]["out"]
    return out

